# Initial kernel scaffold
#
"""Trainium2 Bass kernel for nn_CNNQNetwork (dense_cnn).

The reference network applies 7 small convs to a fixed 4x4x16 input with
VALID padding, concatenates the relu'd outputs (3648 features), then a
3-layer MLP (3648 -> 512 -> 128 -> 4).  Because the spatial input is tiny
and fixed, the whole conv+concat stage is one linear map of the flattened
input: combined = relu(x_flat @ Wc.T + bc) with Wc [3648, 256] assembled
on the host from the conv weights.  So the device kernel is a 4-layer MLP:

    256 -> 3648 (relu) -> 512 (relu) -> 128 (relu) -> 4

Sharding: pure data parallel over 8 NeuronCores (4096 samples each),
weights replicated.  Activations are kept feature-major on-chip
(partitions = features, free dim = batch) so every layer is a natural
lhsT.T @ rhs matmul with no on-chip transposes; the host pre-transposes x
and post-transposes the [4, B] output.  Matmuls run in float32r (fp32
operands truncated to ~fp22 at the PE) which streams at full PE rate for
moving dims >= 256, with fp32 PSUM accumulation.
"""

import numpy as np

import concourse.bass as bass
import concourse.mybir as mybir
import concourse.tile as tile
from concourse.bass import ts
from concourse.bass_utils import run_bass_kernel_spmd

N_CORES = 8
B = 32768
B_LOC = B // N_CORES  # 4096
NB = 512  # batch tile (matmul moving dim)
BT = B_LOC // NB  # 8 batch tiles per core
P = 128
F_IN = 256  # 16*4*4 flattened input features
K1 = F_IN // P  # 2
H1 = 3712  # 3648 padded up to 29*128
M1 = H1 // P  # 29
H2 = 512
M2 = H2 // P  # 4
H3 = 128
NA = 4  # num actions

F32 = mybir.dt.float32
F32R = mybir.dt.float32r

KERNELS = [(1, 2), (2, 1), (1, 3), (3, 1), (1, 4), (4, 1), (2, 2)]

_PROGRAM_CACHE = {}


def _build_dense_first_layer(ws, bs):
    """Collapse the 7 convs into one dense [H1, 256] matrix + bias [H1]."""
    Wc = np.zeros((H1, F_IN), np.float32)
    bc = np.zeros((H1,), np.float32)
    off = 0
    for (kh, kw), w, b in zip(KERNELS, ws, bs):
        oh, ow = 5 - kh, 5 - kw
        blk = np.zeros((64, oh, ow, 16, 4, 4), np.float32)
        for pi in range(oh):
            for pj in range(ow):
                blk[:, pi, pj, :, pi : pi + kh, pj : pj + kw] = w
        n = 64 * oh * ow
        Wc[off : off + n] = blk.reshape(n, F_IN)
        bc[off : off + n] = np.repeat(np.asarray(b, np.float32), oh * ow)
        off += n
    assert off == 3648
    return Wc, bc


def _build_program():
    nc = bass.Bass()
    x_d = nc.declare_dram_parameter("x", [K1, P, B_LOC], F32, isOutput=False)
    wct_d = nc.declare_dram_parameter("wct", [K1, P, H1], F32, isOutput=False)
    bc_d = nc.declare_dram_parameter("bc", [P, M1], F32, isOutput=False)
    fw0_d = nc.declare_dram_parameter("fw0t", [M1, P, H2], F32, isOutput=False)
    fb0_d = nc.declare_dram_parameter("fb0", [P, M2], F32, isOutput=False)
    fw1_d = nc.declare_dram_parameter("fw1t", [M2, P, H3], F32, isOutput=False)
    fb1_d = nc.declare_dram_parameter("fb1", [P, 1], F32, isOutput=False)
    fw2_d = nc.declare_dram_parameter("fw2t", [P, NA], F32, isOutput=False)
    fb2_d = nc.declare_dram_parameter("fb2", [NA, 1], F32, isOutput=False)
    out_d = nc.declare_dram_parameter("out", [NA, B_LOC], F32, isOutput=True)

    RELU = mybir.ActivationFunctionType.Relu
    ADD = mybir.AluOpType.add
    MAX = mybir.AluOpType.max

    with tile.TileContext(nc) as tc:
        with (
            tc.tile_pool(name="wpool", bufs=1) as wpool,
            tc.tile_pool(name="xpool", bufs=2) as xpool,
            tc.tile_pool(name="a1pool", bufs=1) as a1pool,
            tc.tile_pool(name="apool", bufs=2) as apool,
            tc.tile_pool(name="opool", bufs=2) as opool,
            tc.tile_pool(name="pspool", bufs=4, space="PSUM") as pspool,
            tc.tile_pool(name="ps4pool", bufs=2, space="PSUM") as ps4pool,
        ):
            # --- load all (replicated) weights once; they stay resident ---
            wc = wpool.tile([P, K1, H1], F32)
            for k in range(K1):
                nc.sync.dma_start(wc[:, k, :], wct_d[k])
            fw0 = wpool.tile([P, M1, H2], F32)
            for m in range(M1):
                nc.sync.dma_start(fw0[:, m, :], fw0_d[m])
            fw1 = wpool.tile([P, M2, H3], F32)
            for m in range(M2):
                nc.sync.dma_start(fw1[:, m, :], fw1_d[m])
            fw2 = wpool.tile([P, NA], F32)
            nc.sync.dma_start(fw2[:], fw2_d[:])
            bc = wpool.tile([P, M1], F32)
            nc.sync.dma_start(bc[:], bc_d[:])
            fb0 = wpool.tile([P, M2], F32)
            nc.sync.dma_start(fb0[:], fb0_d[:])
            fb1 = wpool.tile([P, 1], F32)
            nc.sync.dma_start(fb1[:], fb1_d[:])
            fb2 = wpool.tile([NA, 1], F32)
            nc.sync.dma_start(fb2[:], fb2_d[:])

            for t in range(BT):
                xt = xpool.tile([P, K1, NB], F32, tag="xt")
                for k in range(K1):
                    nc.sync.dma_start(xt[:, k, :], x_d[k, :, ts(t, NB)])

                # L1: a1 = relu(Wc @ x + bc), feature-major [H1, NB]
                a1 = a1pool.tile([P, M1, NB], F32, tag="a1")
                for m in range(M1):
                    ps = pspool.tile([P, NB], F32, tag="ps")
                    for k in range(K1):
                        nc.tensor.matmul(
                            ps[:],
                            wc[:, k, ts(m, P)].bitcast(F32R),
                            xt[:, k, :].bitcast(F32R),
                            start=(k == 0),
                            stop=(k == K1 - 1),
                        )
                    # split bias+relu between DVE and ACT so neither lags PE
                    if m % 2 == 0:
                        nc.vector.tensor_scalar(
                            a1[:, m, :], ps[:], bc[:, m : m + 1], 0.0, ADD, MAX
                        )
                    else:
                        nc.scalar.activation(
                            a1[:, m, :], ps[:], RELU, bias=bc[:, m : m + 1]
                        )

                # L2: a2 = relu(fw0 @ a1 + fb0), [512, NB]
                a2 = apool.tile([P, M2, NB], F32, tag="a2")
                for m in range(M2):
                    ps = pspool.tile([P, NB], F32, tag="ps")
                    for k in range(M1):
                        nc.tensor.matmul(
                            ps[:],
                            fw0[:, k, ts(m, P)].bitcast(F32R),
                            a1[:, k, :].bitcast(F32R),
                            start=(k == 0),
                            stop=(k == M1 - 1),
                        )
                    if m % 2 == 0:
                        nc.vector.tensor_scalar(
                            a2[:, m, :], ps[:], fb0[:, m : m + 1], 0.0, ADD, MAX
                        )
                    else:
                        nc.scalar.activation(
                            a2[:, m, :], ps[:], RELU, bias=fb0[:, m : m + 1]
                        )

                # L3: a3 = relu(fw1 @ a2 + fb1), [128, NB]
                a3 = apool.tile([P, NB], F32, tag="a3")
                ps = pspool.tile([P, NB], F32, tag="ps")
                for k in range(M2):
                    nc.tensor.matmul(
                        ps[:],
                        fw1[:, k, :].bitcast(F32R),
                        a2[:, k, :].bitcast(F32R),
                        start=(k == 0),
                        stop=(k == M2 - 1),
                    )
                nc.scalar.activation(a3[:], ps[:], RELU, bias=fb1[:, 0:1])

                # L4: out = fw2 @ a3 + fb2, [4, NB]
                ps4 = ps4pool.tile([NA, NB], F32, tag="ps4")
                nc.tensor.matmul(
                    ps4[:],
                    fw2[:].bitcast(F32R),
                    a3[:].bitcast(F32R),
                    start=True,
                    stop=True,
                )
                ob = opool.tile([NA, NB], F32, tag="ob")
                nc.vector.tensor_scalar_add(ob[:], ps4[:], fb2[:, 0:1])
                nc.sync.dma_start(out_d[:, ts(t, NB)], ob[:])

    return nc


def kernel(x, w0, b0, w1, b1, w2, b2, w3, b3, w4, b4, w5, b5, w6, b6,
           fw0, fb0, fw1, fb1, fw2, fb2):
    x = np.asarray(x, np.float32).reshape(B, F_IN)
    ws = [np.asarray(w, np.float32) for w in (w0, w1, w2, w3, w4, w5, w6)]
    bs = [np.asarray(b, np.float32) for b in (b0, b1, b2, b3, b4, b5, b6)]
    fw0 = np.asarray(fw0, np.float32)
    fb0 = np.asarray(fb0, np.float32)
    fw1 = np.asarray(fw1, np.float32)
    fb1 = np.asarray(fb1, np.float32)
    fw2 = np.asarray(fw2, np.float32)
    fb2 = np.asarray(fb2, np.float32)

    Wc, bc = _build_dense_first_layer(ws, bs)

    # host-side packing into the DRAM layouts the kernel expects
    wct_h = np.ascontiguousarray(Wc.T).reshape(K1, P, H1)
    bc_h = np.ascontiguousarray(bc.reshape(M1, P).T)
    fw0_pad = np.zeros((H2, H1), np.float32)
    fw0_pad[:, :3648] = fw0
    fw0t_h = np.ascontiguousarray(fw0_pad.T).reshape(M1, P, H2)
    fb0_h = np.ascontiguousarray(fb0.reshape(M2, P).T)
    fw1t_h = np.ascontiguousarray(fw1.T).reshape(M2, P, H3)
    fb1_h = np.ascontiguousarray(fb1.reshape(1, P).T)
    fw2t_h = np.ascontiguousarray(fw2.T)  # [128, 4]
    fb2_h = np.ascontiguousarray(fb2.reshape(1, NA).T)  # [4, 1]

    if "nc" not in _PROGRAM_CACHE:
        _PROGRAM_CACHE["nc"] = _build_program()
    nc = _PROGRAM_CACHE["nc"]

    shared = {
        "wct": wct_h,
        "bc": bc_h,
        "fw0t": fw0t_h,
        "fb0": fb0_h,
        "fw1t": fw1t_h,
        "fb1": fb1_h,
        "fw2t": fw2t_h,
        "fb2": fb2_h,
    }
    in_maps = []
    for i in range(N_CORES):
        shard = np.ascontiguousarray(
            x[i * B_LOC : (i + 1) * B_LOC].T
        ).reshape(K1, P, B_LOC)
        in_maps.append({"x": shard, **shared})

    res = run_bass_kernel_spmd(nc, in_maps, list(range(N_CORES)))
    out = np.concatenate([r["out"] for r in res.results], axis=1)  # [4, B]
    return np.ascontiguousarray(out.T)


# revision 9
# speedup vs baseline: 2.0683x; 2.0683x over previous
"""Trainium2 Bass kernel for nn_CNNQNetwork (dense_cnn).

The reference network applies 7 small convs to a fixed 4x4x16 input with
VALID padding, concatenates the relu'd outputs (3648 features), then a
3-layer MLP (3648 -> 512 -> 128 -> 4).  Because the spatial input is tiny
and fixed, the whole conv+concat stage is one linear map of the flattened
input: combined = relu(x_flat @ Wc.T + bc) with Wc [3648, 256] assembled
on the host from the conv weights.  So the device kernel is a 4-layer MLP:

    256 -> 3648 (relu) -> 512 (relu) -> 128 (relu) -> 4

Sharding: pure data parallel over 8 NeuronCores (4096 samples each),
weights replicated.  Activations are kept feature-major on-chip
(partitions = features, free dim = batch) so every layer is a natural
lhsT.T @ rhs matmul with no on-chip transposes; the host pre-transposes x
and post-transposes the [4, B] output.  Matmuls run in float32r (fp32
operands truncated to ~fp22 at the PE) which streams at full PE rate for
moving dims >= 256, with fp32 PSUM accumulation.
"""

import numpy as np

import concourse.bass as bass
import concourse.bacc as bacc
import concourse.mybir as mybir
import concourse.tile as tile
from concourse.bass import ts
from concourse.bass_utils import run_bass_kernel_spmd

N_CORES = 8
B = 32768
B_LOC = B // N_CORES  # 4096
NB = 512  # batch tile (matmul moving dim)
BT = B_LOC // NB  # 8 batch tiles per core
P = 128
F_IN = 256  # 16*4*4 flattened input features
K1 = F_IN // P  # 2
H1 = 3712  # 3648 padded up to 29*128
M1 = H1 // P  # 29
H2 = 512
M2 = H2 // P  # 4
H3 = 128
NA = 4  # num actions

F32 = mybir.dt.float32
F32R = mybir.dt.float32r

KERNELS = [(1, 2), (2, 1), (1, 3), (3, 1), (1, 4), (4, 1), (2, 2)]

_PROGRAM_CACHE = {}


def _build_dense_first_layer(ws, bs):
    """Collapse the 7 convs into one dense [H1, 256] matrix + bias [H1]."""
    Wc = np.zeros((H1, F_IN), np.float32)
    bc = np.zeros((H1,), np.float32)
    off = 0
    for (kh, kw), w, b in zip(KERNELS, ws, bs):
        oh, ow = 5 - kh, 5 - kw
        blk = np.zeros((64, oh, ow, 16, 4, 4), np.float32)
        for pi in range(oh):
            for pj in range(ow):
                blk[:, pi, pj, :, pi : pi + kh, pj : pj + kw] = w
        n = 64 * oh * ow
        Wc[off : off + n] = blk.reshape(n, F_IN)
        bc[off : off + n] = np.repeat(np.asarray(b, np.float32), oh * ow)
        off += n
    assert off == 3648
    return Wc, bc


def _build_program(repeat=1):
    nc = bacc.Bacc(None, target_bir_lowering=False)
    x_d = nc.declare_dram_parameter("x", [K1, P, B_LOC], F32R, isOutput=False)
    wct_d = nc.declare_dram_parameter("wct", [K1, P, H1], F32R, isOutput=False)
    bc_d = nc.declare_dram_parameter("bc", [P, M1], F32, isOutput=False)
    fw0_d = nc.declare_dram_parameter("fw0t", [M1, P, H2], F32R, isOutput=False)
    fb0_d = nc.declare_dram_parameter("fb0", [P, M2], F32, isOutput=False)
    fw1_d = nc.declare_dram_parameter("fw1t", [M2, P, H3], F32R, isOutput=False)
    fb1_d = nc.declare_dram_parameter("fb1", [P, 1], F32, isOutput=False)
    fw2_d = nc.declare_dram_parameter("fw2t", [P, NA], F32R, isOutput=False)
    fb2_d = nc.declare_dram_parameter("fb2", [NA, 1], F32, isOutput=False)
    out_d = nc.declare_dram_parameter("out", [NA, B_LOC], F32, isOutput=True)

    RELU = mybir.ActivationFunctionType.Relu
    ADD = mybir.AluOpType.add
    MAX = mybir.AluOpType.max

    with tile.TileContext(nc) as tc:
        with (
            tc.tile_pool(name="wpool", bufs=1) as wpool,
            tc.tile_pool(name="xpool", bufs=2) as xpool,
            tc.tile_pool(name="a1pool", bufs=1) as a1pool,
            tc.tile_pool(name="apool", bufs=2) as apool,
            tc.tile_pool(name="opool", bufs=2) as opool,
            tc.tile_pool(name="pspool", bufs=4, space="PSUM") as pspool,
            tc.tile_pool(name="ps4pool", bufs=2, space="PSUM") as ps4pool,
        ):
            # --- load all (replicated) weights once; they stay resident ---
            wc = wpool.tile([P, K1, H1], F32R)
            for k in range(K1):
                nc.sync.dma_start(wc[:, k, :], wct_d[k])
            fw0 = wpool.tile([P, M1, H2], F32R)
            for m in range(M1):
                nc.sync.dma_start(fw0[:, m, :], fw0_d[m])
            fw1 = wpool.tile([P, M2, H3], F32R)
            for m in range(M2):
                nc.sync.dma_start(fw1[:, m, :], fw1_d[m])
            fw2 = wpool.tile([P, NA], F32R)
            nc.sync.dma_start(fw2[:], fw2_d[:])
            bc = wpool.tile([P, M1], F32)
            nc.sync.dma_start(bc[:], bc_d[:])
            fb0 = wpool.tile([P, M2], F32)
            nc.sync.dma_start(fb0[:], fb0_d[:])
            fb1 = wpool.tile([P, 1], F32)
            nc.sync.dma_start(fb1[:], fb1_d[:])
            fb2 = wpool.tile([NA, 1], F32)
            nc.sync.dma_start(fb2[:], fb2_d[:])

            def body():
              for t in range(BT):
                xt = xpool.tile([P, K1, NB], F32R, tag="xt")
                for k in range(K1):
                    nc.sync.dma_start(xt[:, k, :], x_d[k, :, ts(t, NB)])

                # L1: a1 = relu(Wc @ x + bc), feature-major [H1, NB]
                a1 = a1pool.tile([P, M1, NB], F32R, tag="a1")
                for m in range(M1):
                    ps = pspool.tile([P, NB], F32, tag="ps")
                    for k in range(K1):
                        nc.tensor.matmul(
                            ps[:],
                            wc[:, k, ts(m, P)],
                            xt[:, k, :],
                            start=(k == 0),
                            stop=(k == K1 - 1),
                        )
                    # split bias+relu between DVE and ACT so neither lags PE
                    if m % 2 == 0:
                        nc.vector.tensor_scalar(
                            a1[:, m, :], ps[:], bc[:, m : m + 1], 0.0, ADD, MAX
                        )
                    else:
                        nc.scalar.activation(
                            a1[:, m, :], ps[:], RELU, bias=bc[:, m : m + 1]
                        )

                # L2: a2 = relu(fw0 @ a1 + fb0), [512, NB]
                a2 = apool.tile([P, M2, NB], F32R, tag="a2")
                for m in range(M2):
                    ps = pspool.tile([P, NB], F32, tag="ps")
                    for k in range(M1):
                        nc.tensor.matmul(
                            ps[:],
                            fw0[:, k, ts(m, P)],
                            a1[:, k, :],
                            start=(k == 0),
                            stop=(k == M1 - 1),
                        )
                    if m % 2 == 0:
                        nc.vector.tensor_scalar(
                            a2[:, m, :], ps[:], fb0[:, m : m + 1], 0.0, ADD, MAX
                        )
                    else:
                        nc.scalar.activation(
                            a2[:, m, :], ps[:], RELU, bias=fb0[:, m : m + 1]
                        )

                # L3: a3 = relu(fw1 @ a2 + fb1), [128, NB]
                a3 = apool.tile([P, NB], F32R, tag="a3")
                ps = pspool.tile([P, NB], F32, tag="ps")
                for k in range(M2):
                    nc.tensor.matmul(
                        ps[:],
                        fw1[:, k, :],
                        a2[:, k, :],
                        start=(k == 0),
                        stop=(k == M2 - 1),
                    )
                nc.scalar.activation(a3[:], ps[:], RELU, bias=fb1[:, 0:1])

                # L4: out = fw2 @ a3 + fb2, [4, NB]
                ps4 = ps4pool.tile([NA, NB], F32, tag="ps4")
                nc.tensor.matmul(
                    ps4[:],
                    fw2[:],
                    a3[:],
                    start=True,
                    stop=True,
                )
                ob = opool.tile([NA, NB], F32, tag="ob")
                nc.vector.tensor_scalar_add(ob[:], ps4[:], fb2[:, 0:1])
                nc.sync.dma_start(out_d[:, ts(t, NB)], ob[:])

            if repeat == 1:
                body()
            else:
                with tc.For_i(0, repeat, 1):
                    body()

    nc.finalize()
    return nc


def pack_inputs(x, ws, bs, fw0, fb0, fw1, fb1, fw2, fb2):
    """Pack full-problem numpy inputs into the per-core DRAM in_maps."""
    x = np.asarray(x, np.float32).reshape(B, F_IN)
    ws = [np.asarray(w, np.float32) for w in ws]
    bs = [np.asarray(b, np.float32) for b in bs]
    fw0 = np.asarray(fw0, np.float32)
    fb0 = np.asarray(fb0, np.float32)
    fw1 = np.asarray(fw1, np.float32)
    fb1 = np.asarray(fb1, np.float32)
    fw2 = np.asarray(fw2, np.float32)
    fb2 = np.asarray(fb2, np.float32)

    Wc, bc = _build_dense_first_layer(ws, bs)

    # host-side packing into the DRAM layouts the kernel expects
    wct_h = np.ascontiguousarray(Wc.T).reshape(K1, P, H1)
    bc_h = np.ascontiguousarray(bc.reshape(M1, P).T)
    fw0_pad = np.zeros((H2, H1), np.float32)
    fw0_pad[:, :3648] = fw0
    fw0t_h = np.ascontiguousarray(fw0_pad.T).reshape(M1, P, H2)
    fb0_h = np.ascontiguousarray(fb0.reshape(M2, P).T)
    fw1t_h = np.ascontiguousarray(fw1.T).reshape(M2, P, H3)
    fb1_h = np.ascontiguousarray(fb1.reshape(1, P).T)
    fw2t_h = np.ascontiguousarray(fw2.T)  # [128, 4]
    fb2_h = np.ascontiguousarray(fb2.reshape(1, NA).T)  # [4, 1]

    shared = {
        "wct": wct_h,
        "bc": bc_h,
        "fw0t": fw0t_h,
        "fb0": fb0_h,
        "fw1t": fw1t_h,
        "fb1": fb1_h,
        "fw2t": fw2t_h,
        "fb2": fb2_h,
    }
    in_maps = []
    for i in range(N_CORES):
        shard = np.ascontiguousarray(
            x[i * B_LOC : (i + 1) * B_LOC].T
        ).reshape(K1, P, B_LOC)
        in_maps.append({"x": shard, **shared})
    return in_maps


def kernel(x, w0, b0, w1, b1, w2, b2, w3, b3, w4, b4, w5, b5, w6, b6,
           fw0, fb0, fw1, fb1, fw2, fb2):
    in_maps = pack_inputs(
        x, (w0, w1, w2, w3, w4, w5, w6), (b0, b1, b2, b3, b4, b5, b6),
        fw0, fb0, fw1, fb1, fw2, fb2,
    )
    if "nc" not in _PROGRAM_CACHE:
        _PROGRAM_CACHE["nc"] = _build_program()
    nc = _PROGRAM_CACHE["nc"]

    res = run_bass_kernel_spmd(nc, in_maps, list(range(N_CORES)))
    out = np.concatenate([r["out"] for r in res.results], axis=1)  # [4, B]
    return np.ascontiguousarray(out.T)
